# revision 12
# baseline (speedup 1.0000x reference)
"""Self-contained Bass/Trainium2 kernel for nn_Attention (B=4, N=2048, D=1024, H=16, dh=64).

Sharding: 8 cores = (batch b in 0..3) x (head-group g in 0..1, 8 heads each).
Each core computes q/k/v projections for its 8 heads over the full sequence
(no duplicated K/V work), runs attention, and produces a PARTIAL output
projection y_g = ctx_g @ w_out[512g:512g+512].  The host sums the two
head-group partials per batch and adds the bias.  This is the perfect
1/8 FLOP split: 17.2 GFLOP/core.

Numerics: fp16 everywhere (host converts), fp32 PSUM accumulation.  Softmax
is exp-sum-divide without max subtraction (scores are O(1)); row sums come
free from a ones-column appended to V (AV: out = V_ext.T @ at -> [dh+1, q],
row 64 = sums).

Schedule: ACT (exp, 256 x ~1.07us) and PE (~327us of matmuls) run in
near-lockstep.  Per granule (head, qchunk, keytile j): dots(j) -> exp(j) ->
AV(j-4), one single-matmul K/Q-projection fill for the next head pair, and
one chunk of the previous chunk's softmax normalization on the DVE.  Long
DVE ops are chunked (<2us) so projection-psum evacuations never park behind
them in the DVE FIFO and stall the PE.
"""

import sys
import numpy as np

sys.path.insert(0, "/opt/trn_rl_repo")

B, N, DIM = 4, 2048, 1024
HEADS, DH = 16, 64
SCALE = DH ** -0.5  # 0.125
NC = 8
HG = HEADS // 2          # 8 heads per core
HD = HG * DH             # 512 inner dims per core
HALF = N // 2

LAG = 4                  # AV trails exp by this many granules

_compiled = None


def _build():
    import concourse.tile as tile
    from concourse import bacc, mybir

    f32 = mybir.dt.float32
    f16 = mybir.dt.float16
    EXP = mybir.ActivationFunctionType.Exp

    nc = bacc.Bacc("TRN2", target_bir_lowering=False, debug=False, num_devices=NC)

    XT = nc.dram_tensor("xt", (DIM, N), f16, kind="ExternalInput").ap()
    WQKV = nc.dram_tensor("w_qkv", (DIM, 3 * HD), f16, kind="ExternalInput").ap()
    WOUT = nc.dram_tensor("w_out", (HD, DIM), f16, kind="ExternalInput").ap()
    Y = nc.dram_tensor("y", (N, DIM), f16, kind="ExternalOutput").ap()

    CT = DIM // 128   # 8 contraction tiles over input channels
    MT = HD // 128    # 4 dim tiles for each of q,k (dims-major)
    TT = N // 128     # 16 token tiles
    QC = 1024         # queries per exp granule
    NQC = N // QC     # 2
    VW = DH + 1       # 65: v block width incl. ones column

    with tile.TileContext(nc) as tc:
        with tc.tile_pool(name="persist", bufs=1) as persist:
            xsb = persist.tile([128, CT * N], f16, tag="xsb")          # 32KB/part
            wsb = persist.tile([128, CT * 3 * HD], f16, tag="wsb")     # 24KB/part
            wout_sb = persist.tile([128, MT * DIM], f16, tag="wout")   # 8KB/part
            kT = [persist.tile([128, N], f16, tag="kT", bufs=MT, name=f"kT{m}")
                  for m in range(MT)]
            qT = [persist.tile([128, N], f16, tag="qT", bufs=MT, name=f"qT{m}")
                  for m in range(MT)]
            v_ext = [persist.tile([128, HG * VW], f16, tag="vext", bufs=TT,
                                  name=f"vext{t}") for t in range(TT)]
            ctx_n = [persist.tile([128, N], f16, tag="ctxn", bufs=MT,
                                  name=f"ctxn{m}") for m in range(MT)]

            xv = xsb.rearrange("p (t d) -> p t d", d=N)       # [128, CT, N]
            wv = wsb.rearrange("p (t d) -> p t d", d=3 * HD)  # [128, CT, 3*HD]
            wo = wout_sb.rearrange("p (t d) -> p t d", d=DIM)  # [128, MT, DIM]

            # weights on the ACT hwdge queue, x on the SP queue: they
            # transfer concurrently, and w_v (needed by the first projection)
            # leads its queue so the PE starts ~4us in and never goes cold.
            xr = XT.rearrange("(t p) d -> p t d", p=128)
            wr = WQKV.rearrange("(t p) d -> p t d", p=128)
            for blk in (2, 1, 0):  # v, k, q
                nc.scalar.dma_start(wv[:, :, blk * HD:(blk + 1) * HD],
                                    wr[:, :, blk * HD:(blk + 1) * HD])
            nc.scalar.dma_start(wo, WOUT.rearrange("(t p) d -> p t d", p=128))
            for ct in range(CT):
                nc.sync.dma_start(xv[:, ct, :], xr[:, ct, :])

            with tc.tile_pool(name="proj", bufs=1, space="PSUM") as projp, \
                 tc.tile_pool(name="dots", bufs=1, space="PSUM") as dotsp, \
                 tc.tile_pool(name="ctxp", bufs=1, space="PSUM") as ctxpp, \
                 tc.tile_pool(name="stage", bufs=2) as stage, \
                 tc.tile_pool(name="ysbp", bufs=2) as ysbp:

                def proj_dims_unit(dst, m, base, tc_, tag):
                    """8 closures, one matmul each; the last also evacuates."""
                    cell = {}

                    def mk(ct):
                        def emit():
                            if ct == 0:
                                cell["ps"] = projp.tile(
                                    [128, 512], f32, tag="proj", bufs=2,
                                    name=f"pp_{tag}{m}_{tc_}")
                            ps = cell["ps"]
                            nc.tensor.matmul(
                                ps[:],
                                wv[:, ct, base + m * 128:base + (m + 1) * 128],
                                xv[:, ct, tc_ * 512:(tc_ + 1) * 512],
                                start=(ct == 0), stop=(ct == CT - 1))
                            if ct == CT - 1:
                                nc.vector.tensor_copy(
                                    dst[:, tc_ * 512:(tc_ + 1) * 512], ps[:])
                        return emit
                    return [mk(ct) for ct in range(CT)]

                def proj_v(tt):
                    ps = projp.tile([128, 512], f32, tag="proj", bufs=2,
                                    name=f"pp_v{tt}")
                    for ct in range(CT):
                        nc.tensor.matmul(ps[:],
                                         xv[:, ct, tt * 128:(tt + 1) * 128],
                                         wv[:, ct, 2 * HD:3 * HD],
                                         start=(ct == 0), stop=(ct == CT - 1))
                    dst = v_ext[tt].rearrange("p (h c) -> p h c", c=VW)[:, :, 0:DH]
                    nc.vector.tensor_copy(dst, ps.rearrange("p (h c) -> p h c", c=DH))

                # ---------- phase A: V entirely, then K0/Q0 ----------------
                for tt in range(TT):
                    proj_v(tt)
                for t_ in range(4):
                    for f in proj_dims_unit(kT[0], 0, HD, t_, "k"):
                        f()
                for t_ in range(4):
                    for f in proj_dims_unit(qT[0], 0, 0, t_, "q"):
                        f()
                for t in range(TT):
                    ones_col = v_ext[t].rearrange("p (h c) -> p h c", c=VW)[:, :, DH:VW]
                    nc.gpsimd.memset(ones_col, 1.0)

                # ---------- attention, ACT/PE lockstep ---------------------
                dve_fills = []      # chunked normalize work of the previous
                dve_i = 0           # (h, qc), paced into the current loop

                for h in range(HG):
                    m, p = h // 2, h % 2
                    rows = slice(p * 64, (p + 1) * 64)
                    if p == 0:
                        if h // 2 + 1 < MT:
                            nm = h // 2 + 1
                            fills = []
                            for t_ in range(4):
                                fills += proj_dims_unit(kT[nm], nm, HD, t_, "k")
                            for t_ in range(4):
                                fills += proj_dims_unit(qT[nm], nm, 0, t_, "q")
                        else:
                            fills = []
                        fill_i = 0
                        gran = 0
                        slots = 2 * NQC * TT
                    for qc in range(NQC):
                        qsl = slice(qc * QC, (qc + 1) * QC)
                        cps = ctxpp.tile([65, QC], f32, tag="ctx", bufs=1,
                                         name=f"ctx{h}_{qc}")
                        ats = {}

                        def av(j, cps=cps, h=h):
                            for qs in range(QC // 512):
                                nc.tensor.matmul(
                                    cps[:, qs * 512:(qs + 1) * 512],
                                    v_ext[j][:, h * VW:(h + 1) * VW],
                                    ats[j][:, qs * 512:(qs + 1) * 512],
                                    start=(j == 0), stop=(j == TT - 1))

                        for jt in range(TT):
                            dp = dotsp.tile([128, QC], f32, tag="dots", bufs=2,
                                            name=f"d{h}_{qc}_{jt}")
                            for qs in range(QC // 512):
                                nc.tensor.matmul(
                                    dp[:, qs * 512:(qs + 1) * 512],
                                    kT[m][rows, jt * 128:(jt + 1) * 128],
                                    qT[m][rows, qc * QC + qs * 512:
                                          qc * QC + (qs + 1) * 512],
                                    start=True, stop=True)
                            at = stage.tile([128, QC], f16, tag="at",
                                            bufs=LAG + 2,
                                            name=f"at{h}_{qc}_{jt}")
                            nc.scalar.activation(at[:], dp[:], EXP,
                                                 bias=0.0, scale=SCALE)
                            ats[jt] = at
                            if jt >= LAG:
                                av(jt - LAG)
                                del ats[jt - LAG]
                            gran += 1
                            tgt = (gran * len(fills)) // slots
                            while fill_i < min(tgt, len(fills)):
                                fills[fill_i]()
                                fill_i += 1
                            if gran % 2 == 0 and dve_i < len(dve_fills):
                                dve_fills[dve_i]()
                                dve_i += 1
                        for j in range(TT - LAG, TT):
                            av(j)

                        # evacuate ctx psum now (2 chunks, frees the banks);
                        # queue the rest of the normalization as paced chunks.
                        craw = stage.tile([65, QC], f32, tag="craw", bufs=2,
                                          name=f"craw{h}_{qc}")
                        for c_ in range(2):
                            nc.vector.tensor_copy(
                                craw[:, c_ * 512:(c_ + 1) * 512],
                                cps[:, c_ * 512:(c_ + 1) * 512])

                        def norm_chunks(craw=craw, m=m, rows=rows, qsl=qsl,
                                        h=h, qc=qc):
                            rs = stage.tile([1, QC], f32, tag="rs", bufs=2,
                                            name=f"rs{h}_{qc}")
                            rb = stage.tile([64, QC], f32, tag="rb", bufs=2,
                                            name=f"rb{h}_{qc}")
                            out = []
                            for c_ in range(4):
                                sl = slice(c_ * 256, (c_ + 1) * 256)
                                out.append(lambda sl=sl, rs=rs: nc.vector.reciprocal(
                                    rs[:, sl], craw[64:65, sl]))
                            for c_ in range(2):
                                sl = slice(c_ * 512, (c_ + 1) * 512)
                                out.append(lambda sl=sl, rs=rs, rb=rb:
                                           nc.gpsimd.partition_broadcast(
                                               rb[:, sl], rs[0:1, sl]))
                            q0 = qsl.start
                            for c_ in range(2):
                                sl = slice(c_ * 512, (c_ + 1) * 512)
                                dsl = slice(q0 + c_ * 512, q0 + (c_ + 1) * 512)
                                out.append(lambda sl=sl, dsl=dsl, rb=rb:
                                           nc.vector.tensor_mul(
                                               ctx_n[m][rows, dsl],
                                               craw[0:64, sl], rb[:, sl]))
                            return out

                        # drain any leftover normalize chunks, then queue ours
                        while dve_i < len(dve_fills):
                            dve_fills[dve_i]()
                            dve_i += 1
                        dve_fills = norm_chunks()
                        dve_i = 0

                while fill_i < len(fills):
                    fills[fill_i]()
                    fill_i += 1
                while dve_i < len(dve_fills):
                    dve_fills[dve_i]()
                    dve_i += 1

                # ---------- tail: output projection ------------------------
                # tt 0..7 touch only qc0 rows of ctx_n (normalized long ago),
                # so the PE streams them while the DVE finishes h7/qc1.
                cell = {}
                for tt in list(range(TT // 2)) + list(range(TT // 2, TT)):
                    for ec in range(2):
                        yp = projp.tile([128, 512], f32, tag="proj", bufs=2,
                                        name=f"yp{tt}_{ec}")
                        for mm in range(MT):
                            nc.tensor.matmul(
                                yp[:],
                                ctx_n[mm][:, tt * 128:(tt + 1) * 128],
                                wo[:, mm, ec * 512:(ec + 1) * 512],
                                start=(mm == 0), stop=(mm == MT - 1))
                        g4 = (tt // 4) % 2
                        if tt % 4 == 0 and ec == 0:
                            cell[g4] = ysbp.tile([128, 4 * DIM], f16, tag="ys",
                                                 bufs=2, name=f"ys{tt}")
                        ys = cell[g4]
                        nc.vector.tensor_copy(
                            ys[:, (tt % 4) * DIM + ec * 512:
                               (tt % 4) * DIM + (ec + 1) * 512], yp[:])
                        if tt % 4 == 3 and ec == 1:
                            t0 = tt - 3
                            nc.sync.dma_start(
                                Y[t0 * 128:t0 * 128 + 512, :].rearrange(
                                    "(t p) d -> p t d", p=128),
                                ys.rearrange("p (t d) -> p t d", d=DIM))

    nc.compile()
    return nc


def _get_compiled():
    global _compiled
    if _compiled is None:
        _compiled = _build()
    return _compiled


def make_in_maps(x, w_qkv, w_out):
    """Per-core input maps (shared by kernel() and test profiling)."""
    x = np.asarray(x, dtype=np.float32)
    w_qkv = np.asarray(w_qkv, dtype=np.float32)
    w_out = np.asarray(w_out, dtype=np.float32)
    in_maps = []
    xts = [np.ascontiguousarray(x[b].T.astype(np.float16)) for b in range(B)]
    wq_slices = []
    wo_slices = []
    for g in range(2):
        cols = np.concatenate([
            w_qkv[:, 512 * g: 512 * (g + 1)],
            w_qkv[:, 1024 + 512 * g: 1024 + 512 * (g + 1)],
            w_qkv[:, 2048 + 512 * g: 2048 + 512 * (g + 1)],
        ], axis=1).astype(np.float16)
        wq_slices.append(np.ascontiguousarray(cols))
        wo_slices.append(np.ascontiguousarray(
            w_out[512 * g: 512 * (g + 1), :].astype(np.float16)))
    for c in range(NC):
        b, g = divmod(c, 2)
        in_maps.append({"xt": xts[b], "w_qkv": wq_slices[g],
                        "w_out": wo_slices[g]})
    return in_maps


def kernel(x, w_qkv, w_out, b_out):
    from concourse.bass_utils import run_bass_kernel_spmd

    nc = _get_compiled()
    in_maps = make_in_maps(x, w_qkv, w_out)
    res = run_bass_kernel_spmd(nc, in_maps, core_ids=list(range(NC)))

    b_out = np.asarray(b_out, dtype=np.float32)
    out = np.empty((B, N, DIM), dtype=np.float32)
    for b in range(B):
        out[b] = (res.results[2 * b]["y"].astype(np.float32)
                  + res.results[2 * b + 1]["y"].astype(np.float32) + b_out)
    return out


# revision 15
# speedup vs baseline: 1.0272x; 1.0272x over previous
"""Self-contained Bass/Trainium2 kernel for nn_Attention (B=4, N=2048, D=1024, H=16, dh=64).

Sharding: 8 cores = (batch b in 0..3) x (head-group g in 0..1, 8 heads each).
Each core computes q/k/v projections for its 8 heads over the full sequence
(no duplicated K/V work), runs attention, and produces a PARTIAL output
projection y_g = ctx_g @ w_out[512g:512g+512].  The host sums the two
head-group partials per batch and adds the bias.  This is the perfect
1/8 FLOP split: 17.2 GFLOP/core.

Numerics: fp16 everywhere (host converts), fp32 PSUM accumulation.  Softmax
is exp-sum-divide without max subtraction (scores are O(1)); row sums come
free from a ones-column appended to V (AV: out = V_ext.T @ at -> [dh+1, q],
row 64 = sums).

Schedule: ACT (exp, 256 x ~1.07us) and PE (~327us of matmuls) run in
near-lockstep.  Per granule (head, qchunk, keytile j): dots(j) -> exp(j) ->
AV(j-4), one single-matmul K/Q-projection fill for the next head pair, and
one chunk of the previous chunk's softmax normalization on the DVE.  Long
DVE ops are chunked (<2us) so projection-psum evacuations never park behind
them in the DVE FIFO and stall the PE.
"""

import sys
import numpy as np

sys.path.insert(0, "/opt/trn_rl_repo")

B, N, DIM = 4, 2048, 1024
HEADS, DH = 16, 64
SCALE = DH ** -0.5  # 0.125
NC = 8
HG = HEADS // 2          # 8 heads per core
HD = HG * DH             # 512 inner dims per core
HALF = N // 2

LAG = 4                  # AV trails exp by this many granules

_compiled = None


def _build():
    import concourse.tile as tile
    from concourse import bacc, mybir

    f32 = mybir.dt.float32
    f16 = mybir.dt.float16
    EXP = mybir.ActivationFunctionType.Exp

    nc = bacc.Bacc("TRN2", target_bir_lowering=False, debug=False, num_devices=NC)

    XT = nc.dram_tensor("xt", (DIM, N), f16, kind="ExternalInput").ap()
    WQKV = nc.dram_tensor("w_qkv", (DIM, 3 * HD), f16, kind="ExternalInput").ap()
    WOUT = nc.dram_tensor("w_out", (HD, DIM), f16, kind="ExternalInput").ap()
    Y = nc.dram_tensor("y", (N, DIM), f16, kind="ExternalOutput").ap()

    CT = DIM // 128   # 8 contraction tiles over input channels
    MT = HD // 128    # 4 dim tiles for each of q,k (dims-major)
    TT = N // 128     # 16 token tiles
    QC = 1024         # queries per exp granule
    NQC = N // QC     # 2
    VW = DH + 1       # 65: v block width incl. ones column

    with tile.TileContext(nc) as tc:
        with tc.tile_pool(name="persist", bufs=1) as persist:
            xsb = persist.tile([128, CT * N], f16, tag="xsb")          # 32KB/part
            wsb = persist.tile([128, CT * 3 * HD], f16, tag="wsb")     # 24KB/part
            wout_sb = persist.tile([128, MT * DIM], f16, tag="wout")   # 8KB/part
            kT = [persist.tile([128, N], f16, tag="kT", bufs=MT, name=f"kT{m}")
                  for m in range(MT)]
            qT = [persist.tile([128, N], f16, tag="qT", bufs=MT, name=f"qT{m}")
                  for m in range(MT)]
            v_ext = [persist.tile([128, HG * VW], f16, tag="vext", bufs=TT,
                                  name=f"vext{t}") for t in range(TT)]
            ctx_n = [persist.tile([128, N], f16, tag="ctxn", bufs=MT,
                                  name=f"ctxn{m}") for m in range(MT)]

            xv = xsb.rearrange("p (t d) -> p t d", d=N)       # [128, CT, N]
            wv = wsb.rearrange("p (t d) -> p t d", d=3 * HD)  # [128, CT, 3*HD]
            wo = wout_sb.rearrange("p (t d) -> p t d", d=DIM)  # [128, MT, DIM]

            # weights on the ACT hwdge queue, x on the SP queue: they
            # transfer concurrently, and w_v (needed by the first projection)
            # leads its queue so the PE starts ~4us in and never goes cold.
            xr = XT.rearrange("(t p) d -> p t d", p=128)
            wr = WQKV.rearrange("(t p) d -> p t d", p=128)
            for blk in (2, 1, 0):  # v, k, q
                nc.scalar.dma_start(wv[:, :, blk * HD:(blk + 1) * HD],
                                    wr[:, :, blk * HD:(blk + 1) * HD])
            nc.scalar.dma_start(wo, WOUT.rearrange("(t p) d -> p t d", p=128))
            for ct in range(CT):
                nc.sync.dma_start(xv[:, ct, :], xr[:, ct, :])

            with tc.tile_pool(name="proj", bufs=1, space="PSUM") as projp, \
                 tc.tile_pool(name="dots", bufs=1, space="PSUM") as dotsp, \
                 tc.tile_pool(name="ctxp", bufs=1, space="PSUM") as ctxpp, \
                 tc.tile_pool(name="stage", bufs=2) as stage, \
                 tc.tile_pool(name="ysbp", bufs=2) as ysbp:

                def proj_dims_unit(dst, m, base, tc_, tag):
                    """8 closures, one matmul each; the last also evacuates."""
                    cell = {}

                    def mk(ct):
                        def emit():
                            if ct == 0:
                                cell["ps"] = projp.tile(
                                    [128, 512], f32, tag="proj", bufs=2,
                                    name=f"pp_{tag}{m}_{tc_}")
                            ps = cell["ps"]
                            nc.tensor.matmul(
                                ps[:],
                                wv[:, ct, base + m * 128:base + (m + 1) * 128],
                                xv[:, ct, tc_ * 512:(tc_ + 1) * 512],
                                start=(ct == 0), stop=(ct == CT - 1))
                            if ct == CT - 1:
                                nc.vector.tensor_copy(
                                    dst[:, tc_ * 512:(tc_ + 1) * 512], ps[:])
                        return emit
                    return [mk(ct) for ct in range(CT)]

                def proj_v(tt):
                    ps = projp.tile([128, 512], f32, tag="proj", bufs=2,
                                    name=f"pp_v{tt}")
                    for ct in range(CT):
                        nc.tensor.matmul(ps[:],
                                         xv[:, ct, tt * 128:(tt + 1) * 128],
                                         wv[:, ct, 2 * HD:3 * HD],
                                         start=(ct == 0), stop=(ct == CT - 1))
                    dst = v_ext[tt].rearrange("p (h c) -> p h c", c=VW)[:, :, 0:DH]
                    nc.vector.tensor_copy(dst, ps.rearrange("p (h c) -> p h c", c=DH))

                # ---------- phase A: V entirely, then K0/Q0 ----------------
                # throwaway matmuls on the first-landed weight block: keeps
                # the PE densely busy while x tiles stream in, so the HAM
                # un-throttles (1.2 -> 2.4 GHz) before the real work starts.
                warm = projp.tile([128, 512], f32, tag="proj", bufs=2,
                                  name="warm")
                for w_ in range(24):
                    nc.tensor.matmul(warm[:],
                                     wv[:, w_ % CT, 2 * HD:2 * HD + 128],
                                     wv[:, (w_ + 1) % CT, 2 * HD:2 * HD + 512],
                                     start=True, stop=True)
                for tt in range(TT):
                    proj_v(tt)
                for t_ in range(4):
                    for f in proj_dims_unit(kT[0], 0, HD, t_, "k"):
                        f()
                for t_ in range(4):
                    for f in proj_dims_unit(qT[0], 0, 0, t_, "q"):
                        f()
                for t in range(TT):
                    ones_col = v_ext[t].rearrange("p (h c) -> p h c", c=VW)[:, :, DH:VW]
                    nc.gpsimd.memset(ones_col, 1.0)

                # ---------- attention, ACT/PE lockstep ---------------------
                dve_fills = []      # chunked normalize work of the previous
                dve_i = 0           # (h, qc), paced into the current loop

                for h in range(HG):
                    m, p = h // 2, h % 2
                    rows = slice(p * 64, (p + 1) * 64)
                    if p == 0:
                        if h // 2 + 1 < MT:
                            nm = h // 2 + 1
                            fills = []
                            for t_ in range(4):
                                fills += proj_dims_unit(kT[nm], nm, HD, t_, "k")
                            for t_ in range(4):
                                fills += proj_dims_unit(qT[nm], nm, 0, t_, "q")
                        else:
                            fills = []
                        fill_i = 0
                        gran = 0
                        slots = 2 * NQC * TT
                    for qc in range(NQC):
                        qsl = slice(qc * QC, (qc + 1) * QC)
                        cps = ctxpp.tile([65, QC], f32, tag="ctx", bufs=1,
                                         name=f"ctx{h}_{qc}")
                        ats = {}

                        def av(j, cps=cps, h=h):
                            for qs in range(QC // 512):
                                nc.tensor.matmul(
                                    cps[:, qs * 512:(qs + 1) * 512],
                                    v_ext[j][:, h * VW:(h + 1) * VW],
                                    ats[j][:, qs * 512:(qs + 1) * 512],
                                    start=(j == 0), stop=(j == TT - 1))

                        for jt in range(TT):
                            dp = dotsp.tile([128, QC], f32, tag="dots", bufs=2,
                                            name=f"d{h}_{qc}_{jt}")
                            for qs in range(QC // 512):
                                nc.tensor.matmul(
                                    dp[:, qs * 512:(qs + 1) * 512],
                                    kT[m][rows, jt * 128:(jt + 1) * 128],
                                    qT[m][rows, qc * QC + qs * 512:
                                          qc * QC + (qs + 1) * 512],
                                    start=True, stop=True)
                            at = stage.tile([128, QC], f16, tag="at",
                                            bufs=LAG + 2,
                                            name=f"at{h}_{qc}_{jt}")
                            nc.scalar.activation(at[:], dp[:], EXP,
                                                 bias=0.0, scale=SCALE)
                            ats[jt] = at
                            if jt >= LAG:
                                av(jt - LAG)
                                del ats[jt - LAG]
                            gran += 1
                            tgt = (gran * len(fills)) // slots
                            while fill_i < min(tgt, len(fills)):
                                fills[fill_i]()
                                fill_i += 1
                            if gran % 2 == 0 and dve_i < len(dve_fills):
                                dve_fills[dve_i]()
                                dve_i += 1
                        for j in range(TT - LAG, TT):
                            av(j)

                        # evacuate ctx psum now (2 chunks, frees the banks);
                        # queue the rest of the normalization as paced chunks.
                        craw = stage.tile([65, QC], f32, tag="craw", bufs=2,
                                          name=f"craw{h}_{qc}")
                        for c_ in range(2):
                            nc.vector.tensor_copy(
                                craw[:, c_ * 512:(c_ + 1) * 512],
                                cps[:, c_ * 512:(c_ + 1) * 512])

                        def norm_chunks(craw=craw, m=m, rows=rows, qsl=qsl,
                                        h=h, qc=qc):
                            rs = stage.tile([1, QC], f32, tag="rs", bufs=2,
                                            name=f"rs{h}_{qc}")
                            rb = stage.tile([64, QC], f32, tag="rb", bufs=2,
                                            name=f"rb{h}_{qc}")
                            out = []
                            for c_ in range(4):
                                sl = slice(c_ * 256, (c_ + 1) * 256)
                                out.append(lambda sl=sl, rs=rs: nc.vector.reciprocal(
                                    rs[:, sl], craw[64:65, sl]))
                            for c_ in range(2):
                                sl = slice(c_ * 512, (c_ + 1) * 512)
                                out.append(lambda sl=sl, rs=rs, rb=rb:
                                           nc.gpsimd.partition_broadcast(
                                               rb[:, sl], rs[0:1, sl]))
                            q0 = qsl.start
                            for c_ in range(2):
                                sl = slice(c_ * 512, (c_ + 1) * 512)
                                dsl = slice(q0 + c_ * 512, q0 + (c_ + 1) * 512)
                                out.append(lambda sl=sl, dsl=dsl, rb=rb:
                                           nc.vector.tensor_mul(
                                               ctx_n[m][rows, dsl],
                                               craw[0:64, sl], rb[:, sl]))
                            return out

                        # drain any leftover normalize chunks, then queue ours
                        while dve_i < len(dve_fills):
                            dve_fills[dve_i]()
                            dve_i += 1
                        dve_fills = norm_chunks()
                        dve_i = 0

                while fill_i < len(fills):
                    fills[fill_i]()
                    fill_i += 1

                # ---------- tail: output projection ------------------------
                # tt 0..7 touch only qc0 rows of ctx_n (normalized long ago);
                # emit them first so the PE streams while the DVE finishes
                # h7/qc1's normalization (drained in between).  y leaves in
                # 2-tile batches alternating across both hwdge queues.
                cell = {}

                def outproj(tt):
                    for ec in range(2):
                        yp = projp.tile([128, 512], f32, tag="proj", bufs=2,
                                        name=f"yp{tt}_{ec}")
                        for mm in range(MT):
                            nc.tensor.matmul(
                                yp[:],
                                ctx_n[mm][:, tt * 128:(tt + 1) * 128],
                                wo[:, mm, ec * 512:(ec + 1) * 512],
                                start=(mm == 0), stop=(mm == MT - 1))
                        g2 = (tt // 2) % 2
                        if tt % 2 == 0 and ec == 0:
                            cell[g2] = ysbp.tile([128, 2 * DIM], f16, tag="ys",
                                                 bufs=2, name=f"ys{tt}")
                        ys = cell[g2]
                        nc.vector.tensor_copy(
                            ys[:, (tt % 2) * DIM + ec * 512:
                               (tt % 2) * DIM + (ec + 1) * 512], yp[:])
                        if tt % 2 == 1 and ec == 1:
                            t0 = tt - 1
                            eng = nc.sync if (tt // 2) % 2 == 0 else nc.scalar
                            eng.dma_start(
                                Y[t0 * 128:t0 * 128 + 256, :].rearrange(
                                    "(t p) d -> p t d", p=128),
                                ys.rearrange("p (t d) -> p t d", d=DIM))

                for tt in range(TT // 2):
                    outproj(tt)
                while dve_i < len(dve_fills):
                    dve_fills[dve_i]()
                    dve_i += 1
                for tt in range(TT // 2, TT):
                    outproj(tt)

    nc.compile()
    return nc


def _get_compiled():
    global _compiled
    if _compiled is None:
        _compiled = _build()
    return _compiled


def make_in_maps(x, w_qkv, w_out):
    """Per-core input maps (shared by kernel() and test profiling)."""
    x = np.asarray(x, dtype=np.float32)
    w_qkv = np.asarray(w_qkv, dtype=np.float32)
    w_out = np.asarray(w_out, dtype=np.float32)
    in_maps = []
    xts = [np.ascontiguousarray(x[b].T.astype(np.float16)) for b in range(B)]
    wq_slices = []
    wo_slices = []
    for g in range(2):
        cols = np.concatenate([
            w_qkv[:, 512 * g: 512 * (g + 1)],
            w_qkv[:, 1024 + 512 * g: 1024 + 512 * (g + 1)],
            w_qkv[:, 2048 + 512 * g: 2048 + 512 * (g + 1)],
        ], axis=1).astype(np.float16)
        wq_slices.append(np.ascontiguousarray(cols))
        wo_slices.append(np.ascontiguousarray(
            w_out[512 * g: 512 * (g + 1), :].astype(np.float16)))
    for c in range(NC):
        b, g = divmod(c, 2)
        in_maps.append({"xt": xts[b], "w_qkv": wq_slices[g],
                        "w_out": wo_slices[g]})
    return in_maps


def kernel(x, w_qkv, w_out, b_out):
    from concourse.bass_utils import run_bass_kernel_spmd

    nc = _get_compiled()
    in_maps = make_in_maps(x, w_qkv, w_out)
    res = run_bass_kernel_spmd(nc, in_maps, core_ids=list(range(NC)))

    b_out = np.asarray(b_out, dtype=np.float32)
    out = np.empty((B, N, DIM), dtype=np.float32)
    for b in range(B):
        out[b] = (res.results[2 * b]["y"].astype(np.float32)
                  + res.results[2 * b + 1]["y"].astype(np.float32) + b_out)
    return out


# revision 17
# speedup vs baseline: 1.0355x; 1.0081x over previous
"""Self-contained Bass/Trainium2 kernel for nn_Attention (B=4, N=2048, D=1024, H=16, dh=64).

Sharding: 8 cores = (batch b in 0..3) x (head-group g in 0..1, 8 heads each).
Each core computes q/k/v projections for its 8 heads over the full sequence
(no duplicated K/V work), runs attention, and produces a PARTIAL output
projection y_g = ctx_g @ w_out[512g:512g+512].  The host sums the two
head-group partials per batch and adds the bias.  This is the perfect
1/8 FLOP split: 17.2 GFLOP/core.

Numerics: fp16 everywhere (host converts), fp32 PSUM accumulation.  Softmax
is exp-sum-divide without max subtraction (scores are O(1)); row sums come
free from a ones-column appended to V (AV: out = V_ext.T @ at -> [dh+1, q],
row 64 = sums).

Schedule: ACT (exp, 256 x ~1.07us) and PE (~327us of matmuls) run in
near-lockstep.  Per granule (head, qchunk, keytile j): dots(j) -> exp(j) ->
AV(j-4), one single-matmul K/Q-projection fill for the next head pair, and
one chunk of the previous chunk's softmax normalization on the DVE.  Long
DVE ops are chunked (<2us) so projection-psum evacuations never park behind
them in the DVE FIFO and stall the PE.
"""

import sys
import numpy as np

sys.path.insert(0, "/opt/trn_rl_repo")

B, N, DIM = 4, 2048, 1024
HEADS, DH = 16, 64
SCALE = DH ** -0.5  # 0.125
NC = 8
HG = HEADS // 2          # 8 heads per core
HD = HG * DH             # 512 inner dims per core
HALF = N // 2

LAG = 4                  # AV trails exp by this many granules

_compiled = None


def _build():
    import concourse.tile as tile
    from concourse import bacc, mybir

    f32 = mybir.dt.float32
    f16 = mybir.dt.float16
    EXP = mybir.ActivationFunctionType.Exp

    nc = bacc.Bacc("TRN2", target_bir_lowering=False, debug=False, num_devices=NC)

    XT = nc.dram_tensor("xt", (DIM, N), f16, kind="ExternalInput").ap()
    WQKV = nc.dram_tensor("w_qkv", (DIM, 3 * HD), f16, kind="ExternalInput").ap()
    WOUT = nc.dram_tensor("w_out", (HD, DIM), f16, kind="ExternalInput").ap()
    Y = nc.dram_tensor("y", (N, DIM), f16, kind="ExternalOutput").ap()

    CT = DIM // 128   # 8 contraction tiles over input channels
    MT = HD // 128    # 4 dim tiles for each of q,k (dims-major)
    TT = N // 128     # 16 token tiles
    QC = 1024         # queries per exp granule
    NQC = N // QC     # 2
    VW = DH + 1       # 65: v block width incl. ones column

    with tile.TileContext(nc) as tc:
        with tc.tile_pool(name="persist", bufs=1) as persist:
            xsb = persist.tile([128, CT * N], f16, tag="xsb")          # 32KB/part
            wsb = persist.tile([128, CT * 3 * HD], f16, tag="wsb")     # 24KB/part
            wout_sb = persist.tile([128, MT * DIM], f16, tag="wout")   # 8KB/part
            kT = [persist.tile([128, N], f16, tag="kT", bufs=MT, name=f"kT{m}")
                  for m in range(MT)]
            qT = [persist.tile([128, N], f16, tag="qT", bufs=MT, name=f"qT{m}")
                  for m in range(MT)]
            v_ext = [persist.tile([128, HG * VW], f16, tag="vext", bufs=TT,
                                  name=f"vext{t}") for t in range(TT)]
            ctx_n = [persist.tile([128, N], f16, tag="ctxn", bufs=MT,
                                  name=f"ctxn{m}") for m in range(MT)]

            xv = xsb.rearrange("p (t d) -> p t d", d=N)       # [128, CT, N]
            wv = wsb.rearrange("p (t d) -> p t d", d=3 * HD)  # [128, CT, 3*HD]
            wo = wout_sb.rearrange("p (t d) -> p t d", d=DIM)  # [128, MT, DIM]

            # one queue, demand order: w_v feeds the warmups + V projection,
            # x gets the full bandwidth next (it gates everything), and
            # w_k/w_q/w_out stream in during the ~27us of V-projection work.
            xr = XT.rearrange("(t p) d -> p t d", p=128)
            wr = WQKV.rearrange("(t p) d -> p t d", p=128)
            nc.sync.dma_start(wv[:, :, 2 * HD:3 * HD], wr[:, :, 2 * HD:3 * HD])
            for ct in range(CT):
                nc.sync.dma_start(xv[:, ct, :], xr[:, ct, :])
            for blk in (1, 0):  # k, q
                nc.sync.dma_start(wv[:, :, blk * HD:(blk + 1) * HD],
                                  wr[:, :, blk * HD:(blk + 1) * HD])
            nc.sync.dma_start(wo, WOUT.rearrange("(t p) d -> p t d", p=128))

            with tc.tile_pool(name="proj", bufs=1, space="PSUM") as projp, \
                 tc.tile_pool(name="dots", bufs=1, space="PSUM") as dotsp, \
                 tc.tile_pool(name="ctxp", bufs=1, space="PSUM") as ctxpp, \
                 tc.tile_pool(name="stage", bufs=2) as stage, \
                 tc.tile_pool(name="ysbp", bufs=2) as ysbp:

                def proj_dims_unit(dst, m, base, tc_, tag):
                    """8 closures, one matmul each; the last also evacuates."""
                    cell = {}

                    def mk(ct):
                        def emit():
                            if ct == 0:
                                cell["ps"] = projp.tile(
                                    [128, 512], f32, tag="proj", bufs=2,
                                    name=f"pp_{tag}{m}_{tc_}")
                            ps = cell["ps"]
                            nc.tensor.matmul(
                                ps[:],
                                wv[:, ct, base + m * 128:base + (m + 1) * 128],
                                xv[:, ct, tc_ * 512:(tc_ + 1) * 512],
                                start=(ct == 0), stop=(ct == CT - 1))
                            if ct == CT - 1:
                                nc.vector.tensor_copy(
                                    dst[:, tc_ * 512:(tc_ + 1) * 512], ps[:])
                        return emit
                    return [mk(ct) for ct in range(CT)]

                def proj_v(tt):
                    ps = projp.tile([128, 512], f32, tag="proj", bufs=2,
                                    name=f"pp_v{tt}")
                    for ct in range(CT):
                        nc.tensor.matmul(ps[:],
                                         xv[:, ct, tt * 128:(tt + 1) * 128],
                                         wv[:, ct, 2 * HD:3 * HD],
                                         start=(ct == 0), stop=(ct == CT - 1))
                    dst = v_ext[tt].rearrange("p (h c) -> p h c", c=VW)[:, :, 0:DH]
                    nc.vector.tensor_copy(dst, ps.rearrange("p (h c) -> p h c", c=DH))

                # ---------- phase A: V entirely, then K0/Q0 ----------------
                # throwaway matmuls on the first-landed weight block: keeps
                # the PE densely busy while x tiles stream in, so the HAM
                # un-throttles (1.2 -> 2.4 GHz) before the real work starts.
                warm = projp.tile([128, 512], f32, tag="proj", bufs=2,
                                  name="warm")
                for w_ in range(24):
                    nc.tensor.matmul(warm[:],
                                     wv[:, w_ % CT, 2 * HD:2 * HD + 128],
                                     wv[:, (w_ + 1) % CT, 2 * HD:2 * HD + 512],
                                     start=True, stop=True)
                for tt in range(TT):
                    proj_v(tt)
                # K0/Q0 in interleaved unit pairs so each evacuation hides
                # under the partner unit's matmuls (psum tag has 2 buffers)
                k0q0 = ([proj_dims_unit(kT[0], 0, HD, t_, "k") for t_ in range(4)]
                        + [proj_dims_unit(qT[0], 0, 0, t_, "q") for t_ in range(4)])
                for u in range(0, 8, 2):
                    ua, ub = k0q0[u], k0q0[u + 1]
                    for f in ua[0:4]:
                        f()
                    for f in ub[0:4]:
                        f()
                    for f in ua[4:8]:
                        f()
                    for f in ub[4:8]:
                        f()
                for t in range(TT):
                    ones_col = v_ext[t].rearrange("p (h c) -> p h c", c=VW)[:, :, DH:VW]
                    nc.gpsimd.memset(ones_col, 1.0)

                # ---------- attention, ACT/PE lockstep ---------------------
                dve_fills = []      # chunked normalize work of the previous
                dve_i = 0           # (h, qc), paced into the current loop

                for h in range(HG):
                    m, p = h // 2, h % 2
                    rows = slice(p * 64, (p + 1) * 64)
                    if p == 0:
                        if h // 2 + 1 < MT:
                            nm = h // 2 + 1
                            fills = []
                            for t_ in range(4):
                                fills += proj_dims_unit(kT[nm], nm, HD, t_, "k")
                            for t_ in range(4):
                                fills += proj_dims_unit(qT[nm], nm, 0, t_, "q")
                        else:
                            fills = []
                        fill_i = 0
                        gran = 0
                        slots = 2 * NQC * TT
                    for qc in range(NQC):
                        qsl = slice(qc * QC, (qc + 1) * QC)
                        cps = ctxpp.tile([65, QC], f32, tag="ctx", bufs=1,
                                         name=f"ctx{h}_{qc}")
                        ats = {}

                        def av(j, cps=cps, h=h):
                            for qs in range(QC // 512):
                                nc.tensor.matmul(
                                    cps[:, qs * 512:(qs + 1) * 512],
                                    v_ext[j][:, h * VW:(h + 1) * VW],
                                    ats[j][:, qs * 512:(qs + 1) * 512],
                                    start=(j == 0), stop=(j == TT - 1))

                        for jt in range(TT):
                            dp = dotsp.tile([128, QC], f32, tag="dots", bufs=2,
                                            name=f"d{h}_{qc}_{jt}")
                            for qs in range(QC // 512):
                                nc.tensor.matmul(
                                    dp[:, qs * 512:(qs + 1) * 512],
                                    kT[m][rows, jt * 128:(jt + 1) * 128],
                                    qT[m][rows, qc * QC + qs * 512:
                                          qc * QC + (qs + 1) * 512],
                                    start=True, stop=True)
                            at = stage.tile([128, QC], f16, tag="at",
                                            bufs=LAG + 2,
                                            name=f"at{h}_{qc}_{jt}")
                            nc.scalar.activation(at[:], dp[:], EXP,
                                                 bias=0.0, scale=SCALE)
                            ats[jt] = at
                            if jt >= LAG:
                                av(jt - LAG)
                                del ats[jt - LAG]
                            gran += 1
                            tgt = (gran * len(fills)) // slots
                            while fill_i < min(tgt, len(fills)):
                                fills[fill_i]()
                                fill_i += 1
                            if gran % 2 == 0 and dve_i < len(dve_fills):
                                dve_fills[dve_i]()
                                dve_i += 1
                        for j in range(TT - LAG, TT):
                            av(j)

                        # evacuate ctx psum now (2 chunks, frees the banks);
                        # queue the rest of the normalization as paced chunks.
                        craw = stage.tile([65, QC], f32, tag="craw", bufs=2,
                                          name=f"craw{h}_{qc}")
                        for c_ in range(2):
                            nc.vector.tensor_copy(
                                craw[:, c_ * 512:(c_ + 1) * 512],
                                cps[:, c_ * 512:(c_ + 1) * 512])

                        def norm_chunks(craw=craw, m=m, rows=rows, qsl=qsl,
                                        h=h, qc=qc):
                            rs = stage.tile([1, QC], f32, tag="rs", bufs=2,
                                            name=f"rs{h}_{qc}")
                            rb = stage.tile([64, QC], f32, tag="rb", bufs=2,
                                            name=f"rb{h}_{qc}")
                            out = []
                            for c_ in range(4):
                                sl = slice(c_ * 256, (c_ + 1) * 256)
                                out.append(lambda sl=sl, rs=rs: nc.vector.reciprocal(
                                    rs[:, sl], craw[64:65, sl]))
                            for c_ in range(2):
                                sl = slice(c_ * 512, (c_ + 1) * 512)
                                out.append(lambda sl=sl, rs=rs, rb=rb:
                                           nc.gpsimd.partition_broadcast(
                                               rb[:, sl], rs[0:1, sl]))
                            q0 = qsl.start
                            for c_ in range(2):
                                sl = slice(c_ * 512, (c_ + 1) * 512)
                                dsl = slice(q0 + c_ * 512, q0 + (c_ + 1) * 512)
                                out.append(lambda sl=sl, dsl=dsl, rb=rb:
                                           nc.vector.tensor_mul(
                                               ctx_n[m][rows, dsl],
                                               craw[0:64, sl], rb[:, sl]))
                            return out

                        # drain any leftover normalize chunks, then queue ours
                        while dve_i < len(dve_fills):
                            dve_fills[dve_i]()
                            dve_i += 1
                        dve_fills = norm_chunks()
                        dve_i = 0

                while fill_i < len(fills):
                    fills[fill_i]()
                    fill_i += 1

                # ---------- tail: output projection ------------------------
                # tt 0..7 touch only qc0 rows of ctx_n (normalized long ago);
                # emit them first so the PE streams while the DVE finishes
                # h7/qc1's normalization (drained in between).  y leaves in
                # 2-tile batches alternating across both hwdge queues.
                cell = {}

                def outproj(tt):
                    for ec in range(2):
                        yp = projp.tile([128, 512], f32, tag="proj", bufs=2,
                                        name=f"yp{tt}_{ec}")
                        for mm in range(MT):
                            nc.tensor.matmul(
                                yp[:],
                                ctx_n[mm][:, tt * 128:(tt + 1) * 128],
                                wo[:, mm, ec * 512:(ec + 1) * 512],
                                start=(mm == 0), stop=(mm == MT - 1))
                        g2 = (tt // 2) % 2
                        if tt % 2 == 0 and ec == 0:
                            cell[g2] = ysbp.tile([128, 2 * DIM], f16, tag="ys",
                                                 bufs=2, name=f"ys{tt}")
                        ys = cell[g2]
                        nc.vector.tensor_copy(
                            ys[:, (tt % 2) * DIM + ec * 512:
                               (tt % 2) * DIM + (ec + 1) * 512], yp[:])
                        if tt % 2 == 1 and ec == 1:
                            t0 = tt - 1
                            eng = nc.sync if (tt // 2) % 2 == 0 else nc.scalar
                            eng.dma_start(
                                Y[t0 * 128:t0 * 128 + 256, :].rearrange(
                                    "(t p) d -> p t d", p=128),
                                ys.rearrange("p (t d) -> p t d", d=DIM))

                for tt in range(TT // 2):
                    outproj(tt)
                while dve_i < len(dve_fills):
                    dve_fills[dve_i]()
                    dve_i += 1
                for tt in range(TT // 2, TT):
                    outproj(tt)

    nc.compile()
    return nc


def _get_compiled():
    global _compiled
    if _compiled is None:
        _compiled = _build()
    return _compiled


def make_in_maps(x, w_qkv, w_out):
    """Per-core input maps (shared by kernel() and test profiling)."""
    x = np.asarray(x, dtype=np.float32)
    w_qkv = np.asarray(w_qkv, dtype=np.float32)
    w_out = np.asarray(w_out, dtype=np.float32)
    in_maps = []
    xts = [np.ascontiguousarray(x[b].T.astype(np.float16)) for b in range(B)]
    wq_slices = []
    wo_slices = []
    for g in range(2):
        cols = np.concatenate([
            w_qkv[:, 512 * g: 512 * (g + 1)],
            w_qkv[:, 1024 + 512 * g: 1024 + 512 * (g + 1)],
            w_qkv[:, 2048 + 512 * g: 2048 + 512 * (g + 1)],
        ], axis=1).astype(np.float16)
        wq_slices.append(np.ascontiguousarray(cols))
        wo_slices.append(np.ascontiguousarray(
            w_out[512 * g: 512 * (g + 1), :].astype(np.float16)))
    for c in range(NC):
        b, g = divmod(c, 2)
        in_maps.append({"xt": xts[b], "w_qkv": wq_slices[g],
                        "w_out": wo_slices[g]})
    return in_maps


def kernel(x, w_qkv, w_out, b_out):
    from concourse.bass_utils import run_bass_kernel_spmd

    nc = _get_compiled()
    in_maps = make_in_maps(x, w_qkv, w_out)
    res = run_bass_kernel_spmd(nc, in_maps, core_ids=list(range(NC)))

    b_out = np.asarray(b_out, dtype=np.float32)
    out = np.empty((B, N, DIM), dtype=np.float32)
    for b in range(B):
        out[b] = (res.results[2 * b]["y"].astype(np.float32)
                  + res.results[2 * b + 1]["y"].astype(np.float32) + b_out)
    return out


# revision 23
# speedup vs baseline: 1.0356x; 1.0002x over previous
"""Self-contained Bass/Trainium2 kernel for nn_Attention (B=4, N=2048, D=1024, H=16, dh=64).

Sharding: 8 cores = (batch b in 0..3) x (head-group g in 0..1, 8 heads each).
Each core computes q/k/v projections for its 8 heads over the full sequence
(no duplicated K/V work), runs attention, and produces a PARTIAL output
projection y_g = ctx_g @ w_out[512g:512g+512].  The host sums the two
head-group partials per batch and adds the bias.  This is the perfect
1/8 FLOP split: 17.2 GFLOP/core.

Numerics: fp16 everywhere (host converts), fp32 PSUM accumulation.  Softmax
is exp-sum-divide without max subtraction (scores are O(1)); row sums come
free from a ones-column appended to V (AV: out = V_ext.T @ at -> [dh+1, q],
row 64 = sums).

Schedule: ACT (exp, 256 x ~1.07us) and PE (~327us of matmuls) run in
near-lockstep.  Per granule (head, qchunk, keytile j): dots(j) -> exp(j) ->
AV(j-4), one single-matmul K/Q-projection fill for the next head pair, and
one chunk of the previous chunk's softmax normalization on the DVE.  Long
DVE ops are chunked (<2us) so projection-psum evacuations never park behind
them in the DVE FIFO and stall the PE.
"""

import sys
import numpy as np

sys.path.insert(0, "/opt/trn_rl_repo")

B, N, DIM = 4, 2048, 1024
HEADS, DH = 16, 64
SCALE = DH ** -0.5  # 0.125
NC = 8
HG = HEADS // 2          # 8 heads per core
HD = HG * DH             # 512 inner dims per core
HALF = N // 2

LAG = 5                  # AV trails exp by this many granules

_compiled = None


def _build():
    import concourse.tile as tile
    from concourse import bacc, mybir

    f32 = mybir.dt.float32
    f16 = mybir.dt.float16
    EXP = mybir.ActivationFunctionType.Exp

    nc = bacc.Bacc("TRN2", target_bir_lowering=False, debug=False, num_devices=NC)

    XT = nc.dram_tensor("xt", (DIM, N), f16, kind="ExternalInput").ap()
    WQKV = nc.dram_tensor("w_qkv", (DIM, 3 * HD), f16, kind="ExternalInput").ap()
    WOUT = nc.dram_tensor("w_out", (HD, DIM), f16, kind="ExternalInput").ap()
    Y = nc.dram_tensor("y", (N, DIM), f16, kind="ExternalOutput").ap()

    CT = DIM // 128   # 8 contraction tiles over input channels
    MT = HD // 128    # 4 dim tiles for each of q,k (dims-major)
    TT = N // 128     # 16 token tiles
    QC = 1024         # queries per exp granule
    NQC = N // QC     # 2
    VW = DH + 1       # 65: v block width incl. ones column

    with tile.TileContext(nc) as tc:
        with tc.tile_pool(name="persist", bufs=1) as persist:
            xsb = persist.tile([128, CT * N], f16, tag="xsb")          # 32KB/part
            wsb = persist.tile([128, CT * 3 * HD], f16, tag="wsb")     # 24KB/part
            wout_sb = persist.tile([128, MT * DIM], f16, tag="wout")   # 8KB/part
            kT = [persist.tile([128, N], f16, tag="kT", bufs=MT, name=f"kT{m}")
                  for m in range(MT)]
            qT = [persist.tile([128, N], f16, tag="qT", bufs=MT, name=f"qT{m}")
                  for m in range(MT)]
            v_ext = [persist.tile([128, HG * VW], f16, tag="vext", bufs=TT,
                                  name=f"vext{t}") for t in range(TT)]
            ctx_n = [persist.tile([128, N], f16, tag="ctxn", bufs=MT,
                                  name=f"ctxn{m}") for m in range(MT)]

            xv = xsb.rearrange("p (t d) -> p t d", d=N)       # [128, CT, N]
            wv = wsb.rearrange("p (t d) -> p t d", d=3 * HD)  # [128, CT, 3*HD]
            wo = wout_sb.rearrange("p (t d) -> p t d", d=DIM)  # [128, MT, DIM]

            # one queue, demand order: w_v feeds the warmups + V projection,
            # x gets the full bandwidth next (it gates everything), and
            # w_k/w_q/w_out stream in during the ~27us of V-projection work.
            xr = XT.rearrange("(t p) d -> p t d", p=128)
            wr = WQKV.rearrange("(t p) d -> p t d", p=128)
            nc.sync.dma_start(wv[:, :, 2 * HD:3 * HD], wr[:, :, 2 * HD:3 * HD])
            for ct in range(CT):
                nc.sync.dma_start(xv[:, ct, :], xr[:, ct, :])
            for blk in (1, 0):  # k, q
                nc.sync.dma_start(wv[:, :, blk * HD:(blk + 1) * HD],
                                  wr[:, :, blk * HD:(blk + 1) * HD])
            nc.sync.dma_start(wo, WOUT.rearrange("(t p) d -> p t d", p=128))

            with tc.tile_pool(name="proj", bufs=1, space="PSUM") as projp, \
                 tc.tile_pool(name="dots", bufs=1, space="PSUM") as dotsp, \
                 tc.tile_pool(name="ctxp", bufs=1, space="PSUM") as ctxpp, \
                 tc.tile_pool(name="stage", bufs=2) as stage, \
                 tc.tile_pool(name="ysbp", bufs=2) as ysbp:

                def proj_dims_unit(dst, m, base, tc_, tag):
                    """8 closures, one matmul each; the last also evacuates."""
                    cell = {}

                    def mk(ct):
                        def emit():
                            if ct == 0:
                                cell["ps"] = projp.tile(
                                    [128, 512], f32, tag="proj", bufs=2,
                                    name=f"pp_{tag}{m}_{tc_}")
                            ps = cell["ps"]
                            nc.tensor.matmul(
                                ps[:],
                                wv[:, ct, base + m * 128:base + (m + 1) * 128],
                                xv[:, ct, tc_ * 512:(tc_ + 1) * 512],
                                start=(ct == 0), stop=(ct == CT - 1))
                            if ct == CT - 1:
                                nc.vector.tensor_copy(
                                    dst[:, tc_ * 512:(tc_ + 1) * 512], ps[:])
                        return emit
                    return [mk(ct) for ct in range(CT)]

                def proj_v(tt):
                    ps = projp.tile([128, 512], f32, tag="proj", bufs=2,
                                    name=f"pp_v{tt}")
                    for ct in range(CT):
                        nc.tensor.matmul(ps[:],
                                         xv[:, ct, tt * 128:(tt + 1) * 128],
                                         wv[:, ct, 2 * HD:3 * HD],
                                         start=(ct == 0), stop=(ct == CT - 1))
                    dst = v_ext[tt].rearrange("p (h c) -> p h c", c=VW)[:, :, 0:DH]
                    nc.vector.tensor_copy(dst, ps.rearrange("p (h c) -> p h c", c=DH))

                # ---------- phase A: V entirely, then K0/Q0 ----------------
                # throwaway matmuls on the first-landed weight block: keeps
                # the PE densely busy while x tiles stream in, so the HAM
                # un-throttles (1.2 -> 2.4 GHz) before the real work starts.
                warm = projp.tile([128, 512], f32, tag="proj", bufs=2,
                                  name="warm")
                for w_ in range(24):
                    nc.tensor.matmul(warm[:],
                                     wv[:, w_ % CT, 2 * HD:2 * HD + 128],
                                     wv[:, (w_ + 1) % CT, 2 * HD:2 * HD + 512],
                                     start=True, stop=True)
                for tt in range(TT):
                    proj_v(tt)
                # K0/Q0 in interleaved unit pairs so each evacuation hides
                # under the partner unit's matmuls (psum tag has 2 buffers)
                k0q0 = ([proj_dims_unit(kT[0], 0, HD, t_, "k") for t_ in range(4)]
                        + [proj_dims_unit(qT[0], 0, 0, t_, "q") for t_ in range(4)])
                for u in range(0, 8, 2):
                    ua, ub = k0q0[u], k0q0[u + 1]
                    for f in ua[0:4]:
                        f()
                    for f in ub[0:4]:
                        f()
                    for f in ua[4:8]:
                        f()
                    for f in ub[4:8]:
                        f()
                for t in range(TT):
                    ones_col = v_ext[t].rearrange("p (h c) -> p h c", c=VW)[:, :, DH:VW]
                    nc.gpsimd.memset(ones_col, 1.0)

                # ---------- attention, ACT/PE lockstep ---------------------
                dve_fills = []      # chunked normalize work of the previous
                dve_i = 0           # (h, qc), paced into the current loop
                pending = []        # previous chunk's AV flush + ctx evac,
                                    # emitted in this chunk's first granules
                                    # so the PE never parks on exp(15)

                for h in range(HG):
                    m, p = h // 2, h % 2
                    rows = slice(p * 64, (p + 1) * 64)
                    if p == 0:
                        if h // 2 + 1 < MT:
                            nm = h // 2 + 1
                            fills = []
                            for t_ in range(4):
                                fills += proj_dims_unit(kT[nm], nm, HD, t_, "k")
                            for t_ in range(4):
                                fills += proj_dims_unit(qT[nm], nm, 0, t_, "q")
                        else:
                            fills = []
                        fill_i = 0
                        gran = 0
                        slots = 2 * NQC * TT
                    for qc in range(NQC):
                        qsl = slice(qc * QC, (qc + 1) * QC)
                        cps = ctxpp.tile([65, QC], f32, tag="ctx", bufs=1,
                                         name=f"ctx{h}_{qc}")
                        ats = {}

                        def av(j, cps=cps, h=h, ats=ats):
                            for qs in range(QC // 512):
                                nc.tensor.matmul(
                                    cps[:, qs * 512:(qs + 1) * 512],
                                    v_ext[j][:, h * VW:(h + 1) * VW],
                                    ats[j][:, qs * 512:(qs + 1) * 512],
                                    start=(j == 0), stop=(j == TT - 1))

                        for jt in range(TT):
                            dp = dotsp.tile([128, QC], f32, tag="dots", bufs=2,
                                            name=f"d{h}_{qc}_{jt}")
                            for qs in range(QC // 512):
                                nc.tensor.matmul(
                                    dp[:, qs * 512:(qs + 1) * 512],
                                    kT[m][rows, jt * 128:(jt + 1) * 128],
                                    qT[m][rows, qc * QC + qs * 512:
                                          qc * QC + (qs + 1) * 512],
                                    start=True, stop=True)
                            at = stage.tile([128, QC], f16, tag="at",
                                            bufs=LAG + 2,
                                            name=f"at{h}_{qc}_{jt}")
                            nc.scalar.activation(at[:], dp[:], EXP,
                                                 bias=0.0, scale=SCALE)
                            ats[jt] = at
                            if jt < len(pending):
                                pending[jt]()
                            if jt >= LAG:
                                av(jt - LAG)
                                del ats[jt - LAG]
                            gran += 1
                            tgt = (gran * len(fills)) // slots
                            while fill_i < min(tgt, len(fills)):
                                fills[fill_i]()
                                fill_i += 1
                            # norm chunks only after this chunk's pending
                            # (which includes the craw copies they read)
                            if jt >= len(pending) and dve_i < len(dve_fills):
                                dve_fills[dve_i]()
                                dve_i += 1

                        # defer the AV flush + ctx evacuation into the next
                        # chunk's first granules (pending closures) so the PE
                        # doesn't park on exp(15) at the chunk boundary.
                        craw = stage.tile([65, QC], f32, tag="craw", bufs=2,
                                          name=f"craw{h}_{qc}")

                        def mk_flush(js, av=av, ats=ats):
                            def emit():
                                for j in js:
                                    av(j)
                                    del ats[j]
                            return emit

                        def mk_craw(c_, craw=craw, cps=cps):
                            def emit():
                                nc.vector.tensor_copy(
                                    craw[:, c_ * 512:(c_ + 1) * 512],
                                    cps[:, c_ * 512:(c_ + 1) * 512])
                            return emit

                        pending = [mk_flush([TT - 5, TT - 4]),
                                   mk_flush([TT - 3, TT - 2]),
                                   mk_flush([TT - 1]),
                                   mk_craw(0), mk_craw(1)]

                        def norm_chunks(craw=craw, m=m, rows=rows, qsl=qsl,
                                        h=h, qc=qc):
                            rs = stage.tile([1, QC], f32, tag="rs", bufs=2,
                                            name=f"rs{h}_{qc}")
                            rb = stage.tile([64, QC], f32, tag="rb", bufs=2,
                                            name=f"rb{h}_{qc}")
                            out = []
                            for c_ in range(4):
                                sl = slice(c_ * 256, (c_ + 1) * 256)
                                out.append(lambda sl=sl, rs=rs: nc.vector.reciprocal(
                                    rs[:, sl], craw[64:65, sl]))
                            for c_ in range(2):
                                sl = slice(c_ * 512, (c_ + 1) * 512)
                                out.append(lambda sl=sl, rs=rs, rb=rb:
                                           nc.gpsimd.partition_broadcast(
                                               rb[:, sl], rs[0:1, sl]))
                            q0 = qsl.start
                            for c_ in range(2):
                                sl = slice(c_ * 512, (c_ + 1) * 512)
                                dsl = slice(q0 + c_ * 512, q0 + (c_ + 1) * 512)
                                out.append(lambda sl=sl, dsl=dsl, rb=rb:
                                           nc.vector.tensor_mul(
                                               ctx_n[m][rows, dsl],
                                               craw[0:64, sl], rb[:, sl]))
                            return out

                        # drain any leftover normalize chunks, then queue ours
                        while dve_i < len(dve_fills):
                            dve_fills[dve_i]()
                            dve_i += 1
                        dve_fills = norm_chunks()
                        dve_i = 0

                while fill_i < len(fills):
                    fills[fill_i]()
                    fill_i += 1

                # ---------- tail: output projection ------------------------
                # tt 0..7 touch only qc0 rows of ctx_n (normalized long ago);
                # emit them first so the PE streams while the DVE finishes
                # h7/qc1's normalization (drained in between).  y leaves in
                # 2-tile batches alternating across both hwdge queues.
                cell = {}

                def outproj(tt):
                    for ec in range(2):
                        yp = projp.tile([128, 512], f32, tag="proj", bufs=2,
                                        name=f"yp{tt}_{ec}")
                        for mm in range(MT):
                            nc.tensor.matmul(
                                yp[:],
                                ctx_n[mm][:, tt * 128:(tt + 1) * 128],
                                wo[:, mm, ec * 512:(ec + 1) * 512],
                                start=(mm == 0), stop=(mm == MT - 1))
                        g2 = (tt // 2) % 2
                        if tt % 2 == 0 and ec == 0:
                            cell[g2] = ysbp.tile([128, 2 * DIM], f16, tag="ys",
                                                 bufs=2, name=f"ys{tt}")
                        ys = cell[g2]
                        nc.vector.tensor_copy(
                            ys[:, (tt % 2) * DIM + ec * 512:
                               (tt % 2) * DIM + (ec + 1) * 512], yp[:])
                        if tt % 2 == 1 and ec == 1:
                            t0 = tt - 1
                            eng = nc.sync if (tt // 2) % 2 == 0 else nc.scalar
                            eng.dma_start(
                                Y[t0 * 128:t0 * 128 + 256, :].rearrange(
                                    "(t p) d -> p t d", p=128),
                                ys.rearrange("p (t d) -> p t d", d=DIM))

                outproj(0)
                outproj(1)
                for f in pending:  # last chunk's AV flush + ctx evacuation
                    f()
                for tt in range(2, TT // 2):
                    outproj(tt)
                while dve_i < len(dve_fills):
                    dve_fills[dve_i]()
                    dve_i += 1
                for tt in range(TT // 2, TT):
                    outproj(tt)

    nc.compile()
    return nc


def _get_compiled():
    global _compiled
    if _compiled is None:
        _compiled = _build()
    return _compiled


def make_in_maps(x, w_qkv, w_out):
    """Per-core input maps (shared by kernel() and test profiling)."""
    x = np.asarray(x, dtype=np.float32)
    w_qkv = np.asarray(w_qkv, dtype=np.float32)
    w_out = np.asarray(w_out, dtype=np.float32)
    in_maps = []
    xts = [np.ascontiguousarray(x[b].T.astype(np.float16)) for b in range(B)]
    wq_slices = []
    wo_slices = []
    for g in range(2):
        cols = np.concatenate([
            w_qkv[:, 512 * g: 512 * (g + 1)],
            w_qkv[:, 1024 + 512 * g: 1024 + 512 * (g + 1)],
            w_qkv[:, 2048 + 512 * g: 2048 + 512 * (g + 1)],
        ], axis=1).astype(np.float16)
        wq_slices.append(np.ascontiguousarray(cols))
        wo_slices.append(np.ascontiguousarray(
            w_out[512 * g: 512 * (g + 1), :].astype(np.float16)))
    for c in range(NC):
        b, g = divmod(c, 2)
        in_maps.append({"xt": xts[b], "w_qkv": wq_slices[g],
                        "w_out": wo_slices[g]})
    return in_maps


def kernel(x, w_qkv, w_out, b_out):
    from concourse.bass_utils import run_bass_kernel_spmd

    nc = _get_compiled()
    in_maps = make_in_maps(x, w_qkv, w_out)
    res = run_bass_kernel_spmd(nc, in_maps, core_ids=list(range(NC)))

    b_out = np.asarray(b_out, dtype=np.float32)
    out = np.empty((B, N, DIM), dtype=np.float32)
    for b in range(B):
        out[b] = (res.results[2 * b]["y"].astype(np.float32)
                  + res.results[2 * b + 1]["y"].astype(np.float32) + b_out)
    return out


# revision 26
# speedup vs baseline: 1.0434x; 1.0075x over previous
"""Self-contained Bass/Trainium2 kernel for nn_Attention (B=4, N=2048, D=1024, H=16, dh=64).

Sharding: 8 cores = (batch b in 0..3) x (head-group g in 0..1, 8 heads each).
Each core computes q/k/v projections for its 8 heads over the full sequence
(no duplicated K/V work), runs attention, and produces a PARTIAL output
projection y_g = ctx_g @ w_out[512g:512g+512].  The host sums the two
head-group partials per batch and adds the bias.  This is the perfect
1/8 FLOP split: 17.2 GFLOP/core.

Numerics: fp16 everywhere (host converts), fp32 PSUM accumulation.  Softmax
is exp-sum-divide without max subtraction (scores are O(1)); row sums come
free from a ones-column appended to V (AV: out = V_ext.T @ at -> [dh+1, q],
row 64 = sums).

Schedule: ACT (exp, 256 x ~1.07us) and PE (~327us of matmuls) run in
near-lockstep.  Per granule (head, qchunk, keytile j): dots(j) -> exp(j) ->
AV(j-4), one single-matmul K/Q-projection fill for the next head pair, and
one chunk of the previous chunk's softmax normalization on the DVE.  Long
DVE ops are chunked (<2us) so projection-psum evacuations never park behind
them in the DVE FIFO and stall the PE.
"""

import sys
import numpy as np

sys.path.insert(0, "/opt/trn_rl_repo")

B, N, DIM = 4, 2048, 1024
HEADS, DH = 16, 64
SCALE = DH ** -0.5  # 0.125
NC = 8
HG = HEADS // 2          # 8 heads per core
HD = HG * DH             # 512 inner dims per core
HALF = N // 2

LAG = 5                  # AV trails exp by this many granules

_compiled = None


def _build():
    import concourse.tile as tile
    from concourse import bacc, mybir

    f32 = mybir.dt.float32
    f16 = mybir.dt.float16
    EXP = mybir.ActivationFunctionType.Exp

    nc = bacc.Bacc("TRN2", target_bir_lowering=False, debug=False, num_devices=NC)

    XT = nc.dram_tensor("xt", (DIM, N), f16, kind="ExternalInput").ap()
    WQKV = nc.dram_tensor("w_qkv", (DIM, 3 * HD), f16, kind="ExternalInput").ap()
    WOUT = nc.dram_tensor("w_out", (HD, DIM), f16, kind="ExternalInput").ap()
    Y = nc.dram_tensor("y", (N, DIM), f16, kind="ExternalOutput").ap()

    CT = DIM // 128   # 8 contraction tiles over input channels
    MT = HD // 128    # 4 dim tiles for each of q,k (dims-major)
    TT = N // 128     # 16 token tiles
    QC = 1024         # queries per exp granule
    NQC = N // QC     # 2
    VW = DH + 1       # 65: v block width incl. ones column

    with tile.TileContext(nc) as tc:
        with tc.tile_pool(name="persist", bufs=1) as persist:
            xsb = persist.tile([128, CT * N], f16, tag="xsb")          # 32KB/part
            wsb = persist.tile([128, CT * 3 * HD], f16, tag="wsb")     # 24KB/part
            wout_sb = persist.tile([128, MT * DIM], f16, tag="wout")   # 8KB/part
            kT = [persist.tile([128, N], f16, tag="kT", bufs=MT, name=f"kT{m}")
                  for m in range(MT)]
            qT = [persist.tile([128, N], f16, tag="qT", bufs=MT, name=f"qT{m}")
                  for m in range(MT)]
            v_ext = [persist.tile([128, HG * VW], f16, tag="vext", bufs=TT,
                                  name=f"vext{t}") for t in range(TT)]
            ctx_n = [persist.tile([128, N], f16, tag="ctxn", bufs=MT,
                                  name=f"ctxn{m}") for m in range(MT)]

            xv = xsb.rearrange("p (t d) -> p t d", d=N)       # [128, CT, N]
            wv = wsb.rearrange("p (t d) -> p t d", d=3 * HD)  # [128, CT, 3*HD]
            wo = wout_sb.rearrange("p (t d) -> p t d", d=DIM)  # [128, MT, DIM]

            # one queue, demand order: w_v feeds the warmups + V projection,
            # x gets the full bandwidth next (it gates everything), and
            # w_k/w_q/w_out stream in during the ~27us of V-projection work.
            xr = XT.rearrange("(t p) d -> p t d", p=128)
            wr = WQKV.rearrange("(t p) d -> p t d", p=128)
            nc.sync.dma_start(wv[:, :, 2 * HD:3 * HD], wr[:, :, 2 * HD:3 * HD])
            for ct in range(CT):
                nc.sync.dma_start(xv[:, ct, :], xr[:, ct, :])
            for blk in (1, 0):  # k, q
                nc.sync.dma_start(wv[:, :, blk * HD:(blk + 1) * HD],
                                  wr[:, :, blk * HD:(blk + 1) * HD])
            nc.sync.dma_start(wo, WOUT.rearrange("(t p) d -> p t d", p=128))

            with tc.tile_pool(name="proj", bufs=1, space="PSUM") as projp, \
                 tc.tile_pool(name="dots", bufs=1, space="PSUM") as dotsp, \
                 tc.tile_pool(name="ctxp", bufs=1, space="PSUM") as ctxpp, \
                 tc.tile_pool(name="stage", bufs=2) as stage, \
                 tc.tile_pool(name="ysbp", bufs=2) as ysbp:

                def proj_dims_unit(dst, m, base, tc_, tag):
                    """8 closures, one matmul each; the last also evacuates."""
                    cell = {}

                    def mk(ct):
                        def emit():
                            if ct == 0:
                                cell["ps"] = projp.tile(
                                    [128, 512], f32, tag="proj", bufs=2,
                                    name=f"pp_{tag}{m}_{tc_}")
                            ps = cell["ps"]
                            nc.tensor.matmul(
                                ps[:],
                                wv[:, ct, base + m * 128:base + (m + 1) * 128],
                                xv[:, ct, tc_ * 512:(tc_ + 1) * 512],
                                start=(ct == 0), stop=(ct == CT - 1))
                            if ct == CT - 1:
                                nc.vector.tensor_copy(
                                    dst[:, tc_ * 512:(tc_ + 1) * 512], ps[:])
                        return emit
                    return [mk(ct) for ct in range(CT)]

                def proj_v(tt):
                    ps = projp.tile([128, 512], f32, tag="proj", bufs=2,
                                    name=f"pp_v{tt}")
                    for ct in range(CT):
                        nc.tensor.matmul(ps[:],
                                         xv[:, ct, tt * 128:(tt + 1) * 128],
                                         wv[:, ct, 2 * HD:3 * HD],
                                         start=(ct == 0), stop=(ct == CT - 1))
                    dst = v_ext[tt].rearrange("p (h c) -> p h c", c=VW)[:, :, 0:DH]
                    nc.vector.tensor_copy(dst, ps.rearrange("p (h c) -> p h c", c=DH))

                # ---------- phase A: V entirely, then K0/Q0 ----------------
                # throwaway matmuls on the first-landed weight block: keeps
                # the PE densely busy while x tiles stream in, so the HAM
                # un-throttles (1.2 -> 2.4 GHz) before the real work starts.
                warm = projp.tile([128, 512], f32, tag="proj", bufs=2,
                                  name="warm")
                for w_ in range(24):
                    nc.tensor.matmul(warm[:],
                                     wv[:, w_ % CT, 2 * HD:2 * HD + 128],
                                     wv[:, (w_ + 1) % CT, 2 * HD:2 * HD + 512],
                                     start=True, stop=True)
                for tt in range(TT):
                    proj_v(tt)
                # K0/Q0 in interleaved unit pairs so each evacuation hides
                # under the partner unit's matmuls (psum tag has 2 buffers)
                k0q0 = ([proj_dims_unit(kT[0], 0, HD, t_, "k") for t_ in range(4)]
                        + [proj_dims_unit(qT[0], 0, 0, t_, "q") for t_ in range(4)])
                for u in range(0, 8, 2):
                    ua, ub = k0q0[u], k0q0[u + 1]
                    for f in ua[0:4]:
                        f()
                    for f in ub[0:4]:
                        f()
                    for f in ua[4:8]:
                        f()
                    for f in ub[4:8]:
                        f()
                for t in range(TT):
                    ones_col = v_ext[t].rearrange("p (h c) -> p h c", c=VW)[:, :, DH:VW]
                    nc.gpsimd.memset(ones_col, 1.0)

                # ---------- attention, ACT/PE lockstep ---------------------
                dve_fills = []      # chunked normalize work of the previous
                dve_i = 0           # (h, qc), paced into the current loop
                pending = []        # previous chunk's AV flush + ctx evac,
                                    # emitted in this chunk's first granules
                                    # so the PE never parks on exp(15)

                for h in range(HG):
                    m, p = h // 2, h % 2
                    rows = slice(p * 64, (p + 1) * 64)
                    if p == 0:
                        if h // 2 + 1 < MT:
                            nm = h // 2 + 1
                            fills = []
                            for t_ in range(4):
                                fills += proj_dims_unit(kT[nm], nm, HD, t_, "k")
                            for t_ in range(4):
                                fills += proj_dims_unit(qT[nm], nm, 0, t_, "q")
                        else:
                            fills = []
                        fill_i = 0
                        gran = 0
                        slots = 2 * NQC * TT
                    # last head: qc1 first, so the tail's tt8-15 output
                    # projection (which reads qc1 ctx) never waits on the
                    # final (qc0) normalization
                    qcs = (1, 0) if h == HG - 1 else (0, 1)
                    for qc in qcs:
                        qsl = slice(qc * QC, (qc + 1) * QC)
                        cps = ctxpp.tile([65, QC], f32, tag="ctx", bufs=1,
                                         name=f"ctx{h}_{qc}")
                        ats = {}

                        def av(j, cps=cps, h=h, ats=ats):
                            for qs in range(QC // 512):
                                nc.tensor.matmul(
                                    cps[:, qs * 512:(qs + 1) * 512],
                                    v_ext[j][:, h * VW:(h + 1) * VW],
                                    ats[j][:, qs * 512:(qs + 1) * 512],
                                    start=(j == 0), stop=(j == TT - 1))

                        for jt in range(TT):
                            dp = dotsp.tile([128, QC], f32, tag="dots", bufs=2,
                                            name=f"d{h}_{qc}_{jt}")
                            for qs in range(QC // 512):
                                nc.tensor.matmul(
                                    dp[:, qs * 512:(qs + 1) * 512],
                                    kT[m][rows, jt * 128:(jt + 1) * 128],
                                    qT[m][rows, qc * QC + qs * 512:
                                          qc * QC + (qs + 1) * 512],
                                    start=True, stop=True)
                            at = stage.tile([128, QC], f16, tag="at",
                                            bufs=LAG + 2,
                                            name=f"at{h}_{qc}_{jt}")
                            nc.scalar.activation(at[:], dp[:], EXP,
                                                 bias=0.0, scale=SCALE)
                            ats[jt] = at
                            if jt < len(pending):
                                pending[jt]()
                            if jt >= LAG:
                                av(jt - LAG)
                                del ats[jt - LAG]
                            gran += 1
                            tgt = (gran * len(fills)) // slots
                            while fill_i < min(tgt, len(fills)):
                                fills[fill_i]()
                                fill_i += 1
                            # norm chunks only after this chunk's pending
                            # (which includes the craw copies they read)
                            if jt >= len(pending) and dve_i < len(dve_fills):
                                dve_fills[dve_i]()
                                dve_i += 1

                        # defer the AV flush + ctx evacuation into the next
                        # chunk's first granules (pending closures) so the PE
                        # doesn't park on exp(15) at the chunk boundary.
                        craw = stage.tile([65, QC], f32, tag="craw", bufs=2,
                                          name=f"craw{h}_{qc}")

                        def mk_flush(js, av=av, ats=ats):
                            def emit():
                                for j in js:
                                    av(j)
                                    del ats[j]
                            return emit

                        def mk_craw(c_, craw=craw, cps=cps):
                            def emit():
                                nc.vector.tensor_copy(
                                    craw[:, c_ * 512:(c_ + 1) * 512],
                                    cps[:, c_ * 512:(c_ + 1) * 512])
                            return emit

                        pending = [mk_flush([TT - 5, TT - 4]),
                                   mk_flush([TT - 3, TT - 2]),
                                   mk_flush([TT - 1]),
                                   mk_craw(0), mk_craw(1)]

                        def norm_chunks(craw=craw, m=m, rows=rows, qsl=qsl,
                                        h=h, qc=qc):
                            rs = stage.tile([1, QC], f32, tag="rs", bufs=2,
                                            name=f"rs{h}_{qc}")
                            rb = stage.tile([64, QC], f32, tag="rb", bufs=2,
                                            name=f"rb{h}_{qc}")
                            out = []
                            for c_ in range(4):
                                sl = slice(c_ * 256, (c_ + 1) * 256)
                                out.append(lambda sl=sl, rs=rs: nc.vector.reciprocal(
                                    rs[:, sl], craw[64:65, sl]))
                            for c_ in range(2):
                                sl = slice(c_ * 512, (c_ + 1) * 512)
                                out.append(lambda sl=sl, rs=rs, rb=rb:
                                           nc.gpsimd.partition_broadcast(
                                               rb[:, sl], rs[0:1, sl]))
                            q0 = qsl.start
                            for c_ in range(2):
                                sl = slice(c_ * 512, (c_ + 1) * 512)
                                dsl = slice(q0 + c_ * 512, q0 + (c_ + 1) * 512)
                                out.append(lambda sl=sl, dsl=dsl, rb=rb:
                                           nc.vector.tensor_mul(
                                               ctx_n[m][rows, dsl],
                                               craw[0:64, sl], rb[:, sl]))
                            return out

                        # drain any leftover normalize chunks, then queue ours
                        while dve_i < len(dve_fills):
                            dve_fills[dve_i]()
                            dve_i += 1
                        dve_fills = norm_chunks()
                        dve_i = 0

                while fill_i < len(fills):
                    fills[fill_i]()
                    fill_i += 1

                # ---------- tail: output projection ------------------------
                # tt 0..7 touch only qc0 rows of ctx_n (normalized long ago);
                # emit them first so the PE streams while the DVE finishes
                # h7/qc1's normalization (drained in between).  y leaves in
                # 2-tile batches alternating across both hwdge queues.
                cell = {}

                def outproj(tt):
                    for ec in range(2):
                        yp = projp.tile([128, 512], f32, tag="proj", bufs=2,
                                        name=f"yp{tt}_{ec}")
                        for mm in range(MT):
                            nc.tensor.matmul(
                                yp[:],
                                ctx_n[mm][:, tt * 128:(tt + 1) * 128],
                                wo[:, mm, ec * 512:(ec + 1) * 512],
                                start=(mm == 0), stop=(mm == MT - 1))
                        g2 = (tt // 2) % 2
                        if tt % 2 == 0 and ec == 0:
                            cell[g2] = ysbp.tile([128, 2 * DIM], f16, tag="ys",
                                                 bufs=2, name=f"ys{tt}")
                        ys = cell[g2]
                        nc.vector.tensor_copy(
                            ys[:, (tt % 2) * DIM + ec * 512:
                               (tt % 2) * DIM + (ec + 1) * 512], yp[:])
                        if tt % 2 == 1 and ec == 1:
                            t0 = tt - 1
                            eng = nc.sync if (tt // 2) % 2 == 0 else nc.scalar
                            eng.dma_start(
                                Y[t0 * 128:t0 * 128 + 256, :].rearrange(
                                    "(t p) d -> p t d", p=128),
                                ys.rearrange("p (t d) -> p t d", d=DIM))

                # tt8-15 read qc1 ctx (already normalized); they stream on
                # the PE while the last chunk (h7/qc0) flushes + normalizes.
                for tt in range(8, 13):
                    outproj(tt)
                for f in pending:  # last chunk's AV flush + ctx evacuation
                    f()
                for tt in range(13, TT):
                    outproj(tt)
                while dve_i < len(dve_fills):
                    dve_fills[dve_i]()
                    dve_i += 1
                for tt in range(0, TT // 2):
                    outproj(tt)

    nc.compile()
    return nc


def _get_compiled():
    global _compiled
    if _compiled is None:
        _compiled = _build()
    return _compiled


def make_in_maps(x, w_qkv, w_out):
    """Per-core input maps (shared by kernel() and test profiling)."""
    x = np.asarray(x, dtype=np.float32)
    w_qkv = np.asarray(w_qkv, dtype=np.float32)
    w_out = np.asarray(w_out, dtype=np.float32)
    in_maps = []
    xts = [np.ascontiguousarray(x[b].T.astype(np.float16)) for b in range(B)]
    wq_slices = []
    wo_slices = []
    for g in range(2):
        cols = np.concatenate([
            w_qkv[:, 512 * g: 512 * (g + 1)],
            w_qkv[:, 1024 + 512 * g: 1024 + 512 * (g + 1)],
            w_qkv[:, 2048 + 512 * g: 2048 + 512 * (g + 1)],
        ], axis=1).astype(np.float16)
        wq_slices.append(np.ascontiguousarray(cols))
        wo_slices.append(np.ascontiguousarray(
            w_out[512 * g: 512 * (g + 1), :].astype(np.float16)))
    for c in range(NC):
        b, g = divmod(c, 2)
        in_maps.append({"xt": xts[b], "w_qkv": wq_slices[g],
                        "w_out": wo_slices[g]})
    return in_maps


def kernel(x, w_qkv, w_out, b_out):
    from concourse.bass_utils import run_bass_kernel_spmd

    nc = _get_compiled()
    in_maps = make_in_maps(x, w_qkv, w_out)
    res = run_bass_kernel_spmd(nc, in_maps, core_ids=list(range(NC)))

    b_out = np.asarray(b_out, dtype=np.float32)
    out = np.empty((B, N, DIM), dtype=np.float32)
    for b in range(B):
        out[b] = (res.results[2 * b]["y"].astype(np.float32)
                  + res.results[2 * b + 1]["y"].astype(np.float32) + b_out)
    return out
